# revision 12
# baseline (speedup 1.0000x reference)
"""Trainium2 Bass kernel for nn_Attention (llama-style attention layer, fp32).

Full inputs in, full output out. 8-way tensor-parallel over heads (4 heads
per core, both batches on every core). v2: all matmul operands in bf16
(f32r matmuls self-load their stationary tile every instruction, ~55ns
unhidden per matmul; bf16 gets fast weight load), V projected directly in
[token, feature] orientation (x chunks stationary, wv moving) so attention
needs no PE transposes, bf16 AllToAll (half wire), and a streamed bf16 wo.
  - merged q/k projections in one pass over x (bf16 matmuls, fp32 PSUM)
  - RoPE fused into projection eviction (pair-swap via f32r perm matmul)
  - per-head attention in [feat, tok] layout, softmax denominator via
    all-ones matmul, normalization on eviction
  - per-head AllToAll (8 cores) redistributes attention output from
    head-sharding to token-sharding, overlapped with later heads
  - output projection streams the full wo (bf16) in two passes so the
    last AllToAll hides under the first 3 heads' contributions
"""
import os
import sys

sys.path.insert(0, "/opt/trn_rl_repo")

import ml_dtypes
import numpy as np

import concourse.bass as bass
import concourse.mybir as mybir
import concourse.tile as tile
from concourse import bacc
from concourse.bass import ds, ts
from concourse.bass_utils import run_bass_kernel_spmd

DIM = 4096
N_HEADS = 32
HEAD_DIM = 128
B, S = 2, 2048
TOK = B * S                   # 4096 global tokens
N_CORES = 8
HPC = N_HEADS // N_CORES      # heads per core = 4
FPC = HPC * HEAD_DIM          # features per core = 512
P = 128
KO = DIM // P                 # 32 k-tiles over the model dim
NSTRIPE = TOK // 1024         # 4 projection stripes of 1024 tokens
SCALE = 1.0 / float(np.sqrt(HEAD_DIM))

f32 = mybir.dt.float32
f32r = mybir.dt.float32r
bf16 = mybir.dt.bfloat16
EXP = mybir.ActivationFunctionType.Exp
COPY = mybir.ActivationFunctionType.Copy
MULT = mybir.AluOpType.mult
ADD = mybir.AluOpType.add

_CACHE = {}


def _cluster_matmul_weights(m):
    """Within maximal runs of sync-free PE instructions, stable-sort
    (LDWEIGHTS, MATMUL) units by first occurrence of their stationary
    tile so repeated-weight matmuls become adjacent (then deduped).

    Safety: units with any sync_info (waits or updates) are immovable
    barriers, so semaphore counter ordering is untouched. PSUM chain
    order is preserved because every chain in this kernel walks its
    weight tiles in first-seen order (stable sort keeps ties in place).
    """
    n_windows = 0
    for fn in m.functions:
        for blk in fn.blocks:
            insts = blk.instructions
            pe_idx = [
                i for i, ins in enumerate(insts)
                if type(ins).__name__ in ("InstLdweights", "InstMatmult")
            ]

            def clean(x):
                si = x.sync_info
                return si is None or (not si.on_wait and not si.on_update)

            units = []  # (positions, weight_key, movable)
            i = 0
            while i < len(pe_idx):
                ins = insts[pe_idx[i]]
                if (
                    type(ins).__name__ == "InstLdweights"
                    and i + 1 < len(pe_idx)
                    and type(insts[pe_idx[i + 1]]).__name__ == "InstMatmult"
                ):
                    mm = insts[pe_idx[i + 1]]
                    units.append((
                        [pe_idx[i], pe_idx[i + 1]],
                        str(ins.ins[0]),
                        clean(ins) and clean(mm),
                    ))
                    i += 2
                elif type(ins).__name__ == "InstLdweights":
                    units.append(([pe_idx[i]], str(ins.ins[0]), False))
                    i += 1
                else:
                    units.append(([pe_idx[i]], str(ins.ins[1]), clean(ins)))
                    i += 1

            out_units = []

            def flush(seg):
                nonlocal n_windows
                if len(seg) > 2:
                    first = {}
                    for u in seg:
                        first.setdefault(u[1], len(first))
                    seg = sorted(seg, key=lambda u: first[u[1]])
                    n_windows += 1
                out_units.extend(seg)

            pend = []
            for u in units:
                if u[2]:
                    pend.append(u)
                else:
                    flush(pend)
                    pend = []
                    out_units.append(u)
            flush(pend)

            new_pe = [insts[p] for u in out_units for p in u[0]]
            assert len(new_pe) == len(pe_idx)
            for slot, ins in zip(pe_idx, new_pe):
                insts[slot] = ins
    return n_windows


def _dedup_ldweights(m):
    """Remove InstLdweights that reload the stationary tile already in the
    PE array (same physical access pattern, no intervening weight change).
    The PE engine stream is in-order, so within a block this is exact.
    Only sync-free LDWs are removed; any matmul that self-loads (f32r) or
    any non-deduped LDW updates the tracked weights."""
    removed = 0
    for fn in m.functions:
        for blk in fn.blocks:
            insts = blk.instructions
            cur = None      # weights resident in the PE array
            pending = None  # LDW issued but no matmul consumed it yet
            i = 0
            while i < len(insts):
                inst = insts[i]
                tn = type(inst).__name__
                if tn == "InstLdweights":
                    w = str(inst.ins[0])
                    si = inst.sync_info
                    clean = si is None or (
                        not si.on_wait and not si.on_update
                    )
                    if pending is None and w == cur and clean:
                        del insts[i]
                        removed += 1
                        continue
                    pending = w
                elif tn == "InstMatmult":
                    if pending is not None:
                        cur = pending
                        pending = None
                    else:
                        # self-loading matmul (f32r) replaces array weights
                        cur = str(inst.ins[1])
                i += 1
    return removed


def _build():
    nc = bacc.Bacc(
        "TRN2", target_bir_lowering=False, debug=False, num_devices=N_CORES
    )

    xT = nc.dram_tensor("xT", [DIM, TOK], bf16, kind="ExternalInput")
    # packed weights: per-partition-contiguous tiles (see _prep_inputs)
    wqH = nc.dram_tensor("wqH", [HPC, P, KO, P], bf16, kind="ExternalInput")
    wkH = nc.dram_tensor("wkH", [HPC, P, KO, P], bf16, kind="ExternalInput")
    wvM = nc.dram_tensor("wvM", [KO, P, FPC], bf16, kind="ExternalInput")
    woH = nc.dram_tensor("woH", [DIM // P, P, HPC, N_CORES, P], bf16,
                         kind="ExternalInput")
    cb_d = nc.dram_tensor("cb", [P, S], f32, kind="ExternalInput")
    ss_d = nc.dram_tensor("ss", [P, S], f32, kind="ExternalInput")
    perm_d = nc.dram_tensor("perm", [P, P], f32r, kind="ExternalInput")
    ones_d = nc.dram_tensor("ones", [P, P], bf16, kind="ExternalInput")
    out_e = nc.dram_tensor("out", [DIM, TOK // N_CORES], f32, kind="ExternalOutput")

    xT3 = xT.ap().rearrange("(ko p) t -> p ko t", p=P)       # [128, 32, 4096]
    oe3 = out_e.ap().rearrange("(no p) t -> p no t", p=P)    # [128, 32, 512]

    with tile.TileContext(nc) as tc:
        with tc.tile_pool(name="dram", bufs=1, space="DRAM") as drp, \
             tc.tile_pool(name="const", bufs=1) as constp:
            q_d = drp.tile([FPC, TOK], bf16, tag="q_d", name="q_d")
            k_d = drp.tile([FPC, TOK], bf16, tag="k_d", name="k_d")
            # v in [token, head, feat] orientation: [ttile, 128 tok, h, 128 f]
            v_d = drp.tile([TOK // P, P, HPC, P], bf16, tag="v_d", name="v_d")
            cc_in = [
                drp.tile([N_CORES * P, 512], bf16, tag=f"cci{j}", name=f"cci{j}")
                for j in range(HPC)
            ]
            cc_out = [
                drp.tile([N_CORES * P, 512], bf16, tag=f"cco{j}", name=f"cco{j}")
                for j in range(HPC)
            ]

            q3 = q_d[:].rearrange("(mo p) t -> p mo t", p=P)     # [128, 4, 4096]
            k3 = k_d[:].rearrange("(mo p) t -> p mo t", p=P)
            cci3 = [c[:].rearrange("(r p) t -> p r t", p=P) for c in cc_in]
            cco3 = [c[:].rearrange("(g p) t -> p g t", p=P) for c in cc_out]

            perm_sb = constp.tile([P, P], f32r, tag="perm", name="perm_sb")
            nc.sync.dma_start(perm_sb[:], perm_d.ap())
            ones_sb = constp.tile([P, P], bf16, tag="ones", name="ones_sb")
            nc.sync.dma_start(ones_sb[:], ones_d.ap())

            # ---------- Phase 1: merged Q/K/V projections (+RoPE on q,k) ----
            # K-split: each 1024-token stripe runs k-tiles 0..15 into partial
            # SBUF tiles, then k-tiles 16..31; eviction adds the halves. The
            # halved x live-set gives the pool real prefetch slots.
            with tc.tile_pool(name="p1_rope", bufs=1) as ropep, \
                 tc.tile_pool(name="p1_x", bufs=24) as xp, \
                 tc.tile_pool(name="p1_part", bufs=32) as prt, \
                 tc.tile_pool(name="p1_w", bufs=3) as wp, \
                 tc.tile_pool(name="p1_wv", bufs=2) as wvp, \
                 tc.tile_pool(name="p1_t", bufs=3) as tp, \
                 tc.tile_pool(name="p1_ps", bufs=4, space="PSUM") as pp, \
                 tc.tile_pool(name="p1_ps2", bufs=2, space="PSUM") as pp2:
                cb_sb = ropep.tile([P, S], f32, tag="cb", name="cb_sb")
                ss_sb = ropep.tile([P, S], f32, tag="ss", name="ss_sb")
                nc.sync.dma_start(cb_sb[:], cb_d.ap())
                nc.sync.dma_start(ss_sb[:], ss_d.ap())

                projs = ((wqH, q3), (wkH, k3))
                for n in range(NSTRIPE):  # 4 stripes of 1024 tokens
                  with nc.named_scope(f"p1_s{n}"):
                    parts = {}
                    for h2 in range(2):   # contraction halves (k 0-15, 16-31)
                        # first weight tile ahead of x so matmul 0 isn't
                        # stuck behind 16 queued x transfers
                        wt0 = wp.tile([P, KO // 2, P], bf16, tag="wt",
                                      name="wt")
                        nc.sync.dma_start(
                            wt0[:], wqH.ap()[0][:, ds(16 * h2, 16), :]
                        )
                        xs = [
                            xp.tile([P, 1024], bf16, tag="xsl", name="xs")
                            for _ in range(KO // 2)
                        ]
                        for kl in range(KO // 2):
                            nc.sync.dma_start(
                                xs[kl][:], xT3[:, 16 * h2 + kl, ts(n, 1024)]
                            )
                        wvm = wvp.tile([P, KO // 2, FPC], bf16, tag="wvm",
                                       name="wvm")
                        wvs = wvM.ap()[ds(16 * h2, 16), :, :].rearrange(
                            "k p f -> p k f"
                        )
                        for a in range(4):
                            nc.sync.dma_start(
                                wvm[:, ds(4 * a, 4)], wvs[:, ds(4 * a, 4)]
                            )
                        for pi, (wH, dst3) in enumerate(projs):
                            for m in range(HPC):  # 4 feature tiles (heads)
                                ps_a = pp.tile([P, 512], f32, tag="proj",
                                               name="ps_a")
                                ps_b = pp.tile([P, 512], f32, tag="proj",
                                               name="ps_b")
                                if pi == 0 and m == 0:
                                    wt = wt0
                                else:
                                    wt = wp.tile([P, KO // 2, P], bf16,
                                                 tag="wt", name="wt")
                                    nc.sync.dma_start(
                                        wt[:],
                                        wH.ap()[m][:, ds(16 * h2, 16), :],
                                    )
                                for kl in range(KO // 2):
                                    nc.tensor.matmul(
                                        ps_a[:], wt[:, kl], xs[kl][:, 0:512],
                                        start=(kl == 0), stop=(kl == 15),
                                    )
                                    nc.tensor.matmul(
                                        ps_b[:], wt[:, kl],
                                        xs[kl][:, 512:1024],
                                        start=(kl == 0), stop=(kl == 15),
                                    )
                                for ci, pscur in ((0, ps_a), (1, ps_b)):
                                    if h2 == 0:
                                        part = prt.tile([P, 512], f32,
                                                        tag="part", name="part")
                                        nc.scalar.activation(
                                            part[:], pscur[:], COPY
                                        )
                                        parts[(pi, m, ci)] = part
                                        continue
                                    part = parts[(pi, m, ci)]
                                    tok0 = 1024 * n + 512 * ci
                                    rtok = tok0 % S  # rope tables repeat/batch
                                    raw = tp.tile([P, 512], f32r, tag="raw",
                                                  name="raw")
                                    nc.vector.tensor_tensor(
                                        raw[:], pscur[:], part[:], ADD
                                    )
                                    ps_sw = pp2.tile([P, 512], f32,
                                                     tag="permps",
                                                     name="ps_sw")
                                    nc.tensor.matmul(
                                        ps_sw[:], perm_sb[:], raw[:],
                                        start=True, stop=True,
                                    )
                                    qf = tp.tile([P, 512], bf16, tag="qf",
                                                 name="qf")
                                    nc.vector.tensor_tensor(
                                        qf[:], raw[:],
                                        cb_sb[:, ds(rtok, 512)], MULT,
                                    )
                                    tmp = tp.tile([P, 512], f32, tag="tmp",
                                                  name="tmp")
                                    nc.vector.tensor_tensor(
                                        tmp[:], ps_sw[:],
                                        ss_sb[:, ds(rtok, 512)], MULT,
                                    )
                                    nc.vector.tensor_tensor(
                                        qf[:], qf[:], tmp[:], ADD
                                    )
                                    nc.sync.dma_start(
                                        dst3[:, m, ds(tok0, 512)], qf[:]
                                    )
                        # V: x chunks stationary, wv moving -> [tok, feat]
                        for t in range(8):  # 128-token tiles in this stripe
                            ps_v = pp.tile([P, 512], f32, tag="proj",
                                           name="ps_v")
                            for kl in range(KO // 2):
                                nc.tensor.matmul(
                                    ps_v[:], xs[kl][:, ts(t, P)], wvm[:, kl],
                                    start=(kl == 0), stop=(kl == 15),
                                )
                            if h2 == 0:
                                partv = prt.tile([P, 512], f32, tag="part",
                                                 name="partv")
                                nc.scalar.activation(partv[:], ps_v[:], COPY)
                                parts[("v", t)] = partv
                            else:
                                partv = parts[("v", t)]
                                vout = tp.tile([P, 512], bf16, tag="vout",
                                               name="vout")
                                nc.vector.tensor_tensor(
                                    vout[:], ps_v[:], partv[:], ADD
                                )
                                nc.sync.dma_start(
                                    v_d[8 * n + t],
                                    vout[:].rearrange("p (h f) -> p h f", f=P),
                                )

            # ---------- Phase 3: attention per (head, batch) + AllToAll -----
            with tc.tile_pool(name="bridge", bufs=1) as brp:
              # at2[j]: post-AllToAll attention features, resident into phase 4
              at2 = [
                  brp.tile([P, N_CORES, 512], bf16, tag=f"at2_{j}",
                           name=f"at2_{j}")
                  for j in range(HPC)
              ]
              with tc.tile_pool(name="p3_kqv", bufs=3) as hp, \
                 tc.tile_pool(name="p3_exp", bufs=20) as ep, \
                 tc.tile_pool(name="p3_o", bufs=4) as aop, \
                 tc.tile_pool(name="p3_acc", bufs=4) as accp, \
                 tc.tile_pool(name="p3_ps_s", bufs=2, space="PSUM") as sps, \
                 tc.tile_pool(name="p3_ps_o", bufs=2, space="PSUM") as ops, \
                 tc.tile_pool(name="p3_ps_d", bufs=2, space="PSUM") as dps:
                for h in range(HPC):
                  with nc.named_scope(f"attn_h{h}"):
                    for b in range(B):
                        kh = hp.tile([P, S], bf16, tag="kh", name="kh")
                        nc.sync.dma_start(kh[:], k3[:, h, ts(b, S)])
                        qh = hp.tile([P, S], bf16, tag="qh", name="qh")
                        nc.sync.dma_start(qh[:], q3[:, h, ts(b, S)])
                        vh = hp.tile([P, S // P, P], bf16, tag="vh", name="vh")
                        nc.sync.dma_start(
                            vh[:],
                            v_d[:].rearrange("t p h f -> p t h f")[
                                :, ds(16 * b, 16), h, :
                            ],
                        )
                        # qt pairs: both 512-token chunks of a pair share
                        # every kh/vh stationary load (deduped afterwards)
                        for qp in range(2):
                            q0, q1 = 2 * qp, 2 * qp + 1
                            ets = []
                            # acc0/acc1: elementwise sums of the 16 exp
                            # tiles (feeds a single ones-matmul each, so
                            # the softmax denominator costs 1 matmul, not
                            # 16). Accumulated on DVE (A) / GpSimd (B),
                            # which both have slack.
                            acc0 = accp.tile([P, 512], bf16, tag="acc",
                                             name="acc0")
                            acc1 = accp.tile([P, 512], bf16, tag="acc",
                                             name="acc1")
                            for k2 in range(S // P // 2):  # ktok tile pairs
                                psA = sps.tile([P, 1024], f32, tag="s",
                                               name="psA")
                                psB = sps.tile([P, 1024], f32, tag="s",
                                               name="psB")
                                for kk in range(2):
                                    kt = 2 * k2 + kk
                                    nc.tensor.matmul(
                                        psA[:, ts(kk, 512)],
                                        kh[:, ts(kt, P)], qh[:, ts(q0, 512)],
                                        start=True, stop=True,
                                    )
                                    nc.tensor.matmul(
                                        psB[:, ts(kk, 512)],
                                        kh[:, ts(kt, P)], qh[:, ts(q1, 512)],
                                        start=True, stop=True,
                                    )
                                etA = ep.tile([P, 1024], bf16, tag="e",
                                              name="etA")
                                nc.scalar.activation(
                                    etA[:], psA[:], EXP, scale=SCALE
                                )
                                etB = ep.tile([P, 1024], bf16, tag="e",
                                              name="etB")
                                nc.scalar.activation(
                                    etB[:], psB[:], EXP, scale=SCALE
                                )
                                for kk in range(2):
                                    if k2 == 0 and kk == 0:
                                        nc.vector.tensor_copy(
                                            out=acc0[:], in_=etA[:, 0:512]
                                        )
                                        nc.gpsimd.tensor_copy(
                                            out=acc1[:], in_=etB[:, 0:512]
                                        )
                                    else:
                                        nc.vector.tensor_tensor(
                                            acc0[:], acc0[:],
                                            etA[:, ts(kk, 512)], ADD,
                                        )
                                        nc.gpsimd.tensor_tensor(
                                            acc1[:], acc1[:],
                                            etB[:, ts(kk, 512)], ADD,
                                        )
                                ets.append((etA, etB))
                            ps_o0 = ops.tile([P, 512], f32, tag="o",
                                             name="ps_o0")
                            ps_o1 = ops.tile([P, 512], f32, tag="o",
                                             name="ps_o1")
                            with tc.high_priority(offset=-300):
                                for kt in range(S // P):
                                    eA = ets[kt // 2][0][:, ts(kt % 2, 512)]
                                    eB = ets[kt // 2][1][:, ts(kt % 2, 512)]
                                    nc.tensor.matmul(
                                        ps_o0[:], vh[:, kt], eA,
                                        start=(kt == 0),
                                        stop=(kt == S // P - 1),
                                    )
                                    nc.tensor.matmul(
                                        ps_o1[:], vh[:, kt], eB,
                                        start=(kt == 0),
                                        stop=(kt == S // P - 1),
                                    )
                            ps_d0 = dps.tile([P, 512], f32, tag="d",
                                             name="ps_d0")
                            ps_d1 = dps.tile([P, 512], f32, tag="d",
                                             name="ps_d1")
                            nc.tensor.matmul(
                                ps_d0[:], ones_sb[:], acc0[:],
                                start=True, stop=True,
                            )
                            nc.tensor.matmul(
                                ps_d1[:], ones_sb[:], acc1[:],
                                start=True, stop=True,
                            )
                            for qt, ps_o, ps_d in (
                                (q0, ps_o0, ps_d0), (q1, ps_o1, ps_d1)
                            ):
                                rec = aop.tile([P, 512], f32, tag="rec",
                                               name="rec")
                                nc.vector.reciprocal_approx_fast(
                                    rec[:], ps_d[:]
                                )
                                ao = aop.tile([P, 512], bf16, tag="ao",
                                              name="ao")
                                nc.vector.tensor_tensor(
                                    ao[:], ps_o[:], rec[:], MULT
                                )
                                nc.sync.dma_start(
                                    cci3[h][:, 4 * b + qt, :], ao[:]
                                )
                    # all 8 token-chunks of head h written -> redistribute
                    nc.gpsimd.collective_compute(
                        "AllToAll",
                        mybir.AluOpType.bypass,
                        replica_groups=[list(range(N_CORES))],
                        ins=[cc_in[h][:]],
                        outs=[cc_out[h][:]],
                    )
                    # gpsimd ring: a sync-ring copy here would head-of-line
                    # block later w6 prefetches behind the collective wait
                    nc.gpsimd.dma_start(at2[h][:], cco3[h])

              # ---------- Phase 4: output projection (full wo, streamed) ----
              # Two passes: j=0..2 contributions first (only needs the first
              # three AllToAlls -> overlaps the last one), then j=3 + combine.
              with tc.tile_pool(name="p4_w", bufs=4) as wop, \
                   tc.tile_pool(name="p4_w2", bufs=4) as wop2, \
                   tc.tile_pool(name="p4_part", bufs=32) as p4p, \
                   tc.tile_pool(name="p4_s", bufs=4) as osp, \
                   tc.tile_pool(name="p4_ps", bufs=6, space="PSUM") as opp:
                with nc.named_scope("p4_a"):
                  oparts = []
                  for nt in range(DIM // P):  # 32 dout tiles: j = 0..2
                    psum = opp.tile([P, 512], f32, tag="ops", name="psum")
                    w6 = wop.tile([P, HPC - 1, N_CORES, P], bf16, tag="w6",
                                  name="w6")
                    for j in range(HPC - 1):
                        nc.sync.dma_start(w6[:, j], woH.ap()[nt][:, j, :, :])
                    for j in range(HPC - 1):
                        for g in range(N_CORES):
                            nc.tensor.matmul(
                                psum[:], w6[:, j, g], at2[j][:, g],
                                start=(j == 0 and g == 0),
                                stop=(j == HPC - 2 and g == N_CORES - 1),
                            )
                    op = p4p.tile([P, 512], bf16, tag="opart", name="op")
                    nc.scalar.activation(op[:], psum[:], COPY)
                    oparts.append(op)
                with nc.named_scope("p4_b"):
                  for nt in range(DIM // P):  # j = 3 + combine + store
                    psum = opp.tile([P, 512], f32, tag="ops", name="psum")
                    w2 = wop2.tile([P, N_CORES, P], bf16, tag="w2", name="w2")
                    nc.sync.dma_start(w2[:], woH.ap()[nt][:, HPC - 1, :, :])
                    for g in range(N_CORES):
                        nc.tensor.matmul(
                            psum[:], w2[:, g], at2[HPC - 1][:, g],
                            start=(g == 0), stop=(g == N_CORES - 1),
                        )
                    ob = osp.tile([P, 512], f32, tag="ob", name="ob")
                    nc.vector.tensor_tensor(ob[:], psum[:], oparts[nt][:], ADD)
                    nc.sync.dma_start(oe3[:, nt], ob[:])

    nc.compile()
    n_win = _cluster_matmul_weights(nc.m)
    n_removed = _dedup_ldweights(nc.m)
    print(f"[kernel] clustered {n_win} windows, "
          f"deduped {n_removed} redundant LDWEIGHTS")
    return nc


def _prep_inputs(x, freqs_cos, freqs_sin, wq, wk, wv, wo):
    bf = ml_dtypes.bfloat16
    x = np.asarray(x, dtype=np.float32)
    fc = np.asarray(freqs_cos, dtype=np.float32)
    fs = np.asarray(freqs_sin, dtype=np.float32)
    wq = np.asarray(wq, dtype=np.float32)
    wk = np.asarray(wk, dtype=np.float32)
    wv = np.asarray(wv, dtype=np.float32)
    wo = np.asarray(wo, dtype=np.float32)

    cb = np.ascontiguousarray(np.repeat(fc.T, 2, axis=0))  # [128,S]: cos[t,p//2]
    ss = np.repeat(fs.T, 2, axis=0)                        # [128, S]
    ss[0::2, :] *= -1.0                      # even rows: -sin, odd rows: +sin
    ss = np.ascontiguousarray(ss, dtype=np.float32)

    idx = np.arange(P)
    perm = np.zeros((P, P), dtype=np.float32)
    perm[idx ^ 1, idx] = 1.0                 # psum[p, t] = raw[p^1, t]
    ones = np.ones((P, P), dtype=bf)

    xTf = np.ascontiguousarray(x.reshape(TOK, DIM).T.astype(bf))

    def pack_qk(w, rows):
        # [4096 in, 512 out] -> [m 4, p 128, ko 32, mc 128], per-partition
        # contiguous rows
        wT = w[rows].T
        return np.ascontiguousarray(
            wT.reshape(KO, P, HPC, P).transpose(2, 1, 0, 3).astype(bf)
        )

    # wo.T [feat, dout] -> [nt 32, p 128, j 4, g 8, d 128]
    woHf = np.ascontiguousarray(
        wo.T.reshape(N_CORES, HPC, P, DIM // P, P).transpose(3, 2, 1, 0, 4)
        .astype(bf)
    )
    in_maps = []
    for c in range(N_CORES):
        rows = slice(FPC * c, FPC * (c + 1))
        # wv moving layout: [ko 32, p 128 (din), 512 feat]
        wvMf = np.ascontiguousarray(
            wv[rows].T.reshape(KO, P, FPC).astype(bf)
        )
        in_maps.append({
            "xT": xTf,
            "wqH": pack_qk(wq, rows),
            "wkH": pack_qk(wk, rows),
            "wvM": wvMf,
            "woH": woHf,
            "cb": cb,
            "ss": ss,
            "perm": perm,
            "ones": ones,
        })
    return in_maps


def _gather(results):
    y = np.empty((B, S, DIM), dtype=np.float32)
    for c in range(N_CORES):
        b, r = divmod(c, N_CORES // B)
        o = results[c]["out"]  # [4096 dout, 512 tok]
        y[b, 512 * r:512 * (r + 1), :] = o.T
    return y


def kernel(x, start_pos, freqs_cos, freqs_sin, wq, wk, wv, wo, trace=False):
    if "nc" not in _CACHE:
        _CACHE["nc"] = _build()
    nc = _CACHE["nc"]
    in_maps = _prep_inputs(x, freqs_cos, freqs_sin, wq, wk, wv, wo)
    res = run_bass_kernel_spmd(
        nc, in_maps, core_ids=list(range(N_CORES)), trace=trace
    )
    _CACHE["last_result"] = res
    return _gather(res.results)
